# revision 11
# baseline (speedup 1.0000x reference)
"""Trainium2 Bass kernel for DEMA (double exponential moving average) decomposition.

reference semantics (per batch row b, channel c, over time t):
    s0 = x[0], b0 = x[1] - x[0]
    for t in 1..T-1:
        s_t = alpha*x_t + (1-alpha)*(s_{t-1} + b_{t-1})
        b_t = beta*(s_t - s_{t-1}) + (1-beta)*b_{t-1}
    ma = [s0, s1, ..., s_{T-1}];  res = x - ma;  returns (res, ma)

The recurrence is linear in x, so it is restructured into dense matmuls:
time is split into NB=8 blocks of L=96.  With z_t = (s_t, b_t) and
z_t = A z_{t-1} + c x_t (A, c functions of alpha/beta only):

    ma_block0   = W0 @ x_block0
    ma_blockk   = [W | PQ] @ [x_blockk ; Z_k]       (k >= 1)
    Z_(1..7)    = sum_j V_j @ x_blockj              (block-entry states)

W0, [W|PQ], V_j are tiny constant matrices computed on the host in float64.
On-device work is fp32 TensorE matmuls + PSUM evictions + one subtract.
Z_k is copied into 2 extra SBUF partitions (96:98) below each x block so the
state contribution rides in the same matmul (K=98).

Sharding: batch dim (128) split across 8 cores (16 rows each); the
recurrence runs only over time so no cross-core communication is needed.
"""

import numpy as np
from contextlib import ExitStack

import bass_rust as _bass_rust
import concourse.bass as bass
import concourse.tile as tile
import concourse.tile_sem_assignment as _tsa
from concourse import mybir
from concourse.bass_utils import run_bass_kernel_spmd
from concourse.vector_clock import VectorClock, ScopedClock
from concourse.tile_scheduler import N_PROCS

# The walrus build in this container allows at most 2 sync waits per PE
# instruction (S3_LW).  Pin all HWDGE DMAs to a single semaphore lane so any
# consumer needs at most {DMAHW0, <one engine sem>}.  Physical DMA queue
# fan-out is per transfer shape and unaffected by the sem-lane count.
_tsa.NUM_HWDGE_SEMS = 1

N_CORES = 8
B, T, C = 128, 768, 256
L = 96
NB = T // L            # 8 time blocks
BS = B // N_CORES      # 16 batch rows per core
NPAIR = BS // 2        # batch rows processed in pairs
F32 = mybir.dt.float32

# packed weight tensor [98, _WCOLS]: [W0T | WAUGT | VT_0..VT_6]
_WC_W0T = 0            # rows 0:96
_WC_WAUGT = L          # rows 0:98 = [W | PQ].T
_WC_VT = 2 * L         # rows 0:96, 14 cols per j
_WCOLS = 2 * L + 14 * (NB - 1)


def _dema_matrices(alpha: float, beta: float):
    """Block-decomposition coefficient matrices, in float64."""
    a, b = float(alpha), float(beta)
    A = np.array([[1.0 - a, 1.0 - a], [-a * b, b * (1.0 - a) + (1.0 - b)]])
    c = np.array([a, a * b])
    e_s = np.array([1.0, 0.0])

    Apow = [np.eye(2)]
    for _ in range(T + 1):
        Apow.append(A @ Apow[-1])

    # generic block k>=1: z_{kL+t'} = A^{t'+1} Z_k + sum_{i'<=t'} A^{t'-i'} c x[kL+i']
    W = np.zeros((L, L))
    for tp in range(L):
        for ip in range(tp + 1):
            W[tp, ip] = (Apow[tp - ip] @ c)[0]
    PQ = np.zeros((L, 2))
    for tp in range(L):
        PQ[tp, :] = e_s @ Apow[tp + 1]
    U = np.zeros((2, L))
    for ip in range(L):
        U[:, ip] = Apow[L - 1 - ip] @ c
    T2 = Apow[L]

    # block 0: z_0 = (x_0, x_1 - x_0), ma_0 = x_0
    W0 = np.zeros((L, L))
    W0[0, 0] = 1.0
    M0 = np.array([[1.0, 0.0], [-1.0, 1.0]])
    for t in range(1, L):
        zc = Apow[t] @ M0
        W0[t, 0] += (e_s @ zc)[0]
        W0[t, 1] += (e_s @ zc)[1]
        for i in range(1, t + 1):
            W0[t, i] += (Apow[t - i] @ c)[0]
    U0 = np.zeros((2, L))
    zc = Apow[L - 1] @ M0
    U0[:, 0] += zc[:, 0]
    U0[:, 1] += zc[:, 1]
    for i in range(1, L):
        U0[:, i] += Apow[L - 1 - i] @ c

    # V_j: Z_all = sum_j V_j @ x_blockj where Z_k = sum_{j<k} T2^{k-1-j} Uj x_j
    T2pow = [np.eye(2)]
    for _ in range(NB):
        T2pow.append(T2 @ T2pow[-1])
    V = np.zeros((NB - 1, 2 * (NB - 1), L))
    for j in range(NB - 1):
        Uj = U0 if j == 0 else U
        for k in range(j + 1, NB):
            V[j, 2 * (k - 1) : 2 * k, :] = T2pow[k - 1 - j] @ Uj
    return W0, W, PQ, V


def _pack_weights(alpha: float, beta: float) -> np.ndarray:
    W0, W, PQ, V = _dema_matrices(alpha, beta)
    wts = np.zeros((L + 2, _WCOLS), dtype=np.float32)
    wts[0:L, _WC_W0T : _WC_W0T + L] = W0.T
    wts[0:L, _WC_WAUGT : _WC_WAUGT + L] = W.T
    wts[L : L + 2, _WC_WAUGT : _WC_WAUGT + L] = PQ.T
    for j in range(NB - 1):
        wts[0:L, _WC_VT + 14 * j : _WC_VT + 14 * (j + 1)] = V[j].T
    return wts


def _wait_limit(inst) -> int:
    # walrus in this container rejects >1 sync wait on several instruction
    # formats (S3_LW, DMA DIRECT2D, CTRL); keep a single wait everywhere
    return 1


class _SplitDrainTC(tile.TileContext):
    """This walrus build rejects more than a couple of sync waits per
    instruction.  After scheduling + the stock kernel-tail drain, walk every
    block and move excess waits onto injected same-engine nops placed
    immediately before the over-limit instruction (waits execute on the
    engine sequencer before dispatch, so this is semantics-preserving)."""

    def _drain_and_barrier(self, tick_clock, wait_clock):
        super()._drain_and_barrier(tick_clock, wait_clock)
        self._split_excess_waits()

    def _split_excess_waits(self):
        nc = self.nc
        cur_list = nc.cur_bb.bb.instructions if nc.cur_bb is not None else None
        for fn in nc.m.functions:
            for blk in fn.blocks:
                insts = blk.instructions
                i = 0
                while i < len(insts):
                    inst = insts[i]
                    si = getattr(inst, "sync_info", None)
                    waits = list(si.on_wait) if si is not None else []
                    limit = _wait_limit(inst)
                    if len(waits) <= limit:
                        i += 1
                        continue
                    keep = waits[:limit]
                    excess = waits[limit:]
                    nops = []
                    for j in range(0, len(excess)):
                        nop = nc.engines[inst.engine].nop(nofuse=True).ins
                        # engine.nop() appended to the current bb; relocate it
                        if cur_list is not None and cur_list and cur_list[-1] is nop:
                            cur_list.pop()
                        nop.sync_info = _bass_rust.SyncInfo(
                            on_wait=excess[j : j + 1], on_update=[]
                        )
                        nops.append(nop)
                    si.on_wait = keep
                    insts[i:i] = nops
                    i += len(nops) + 1


def _dram_row_ap(dram, b: int):
    """[t'=96 (partition), k=8, c=256] strided view of dram[b]."""
    return bass.AP(
        tensor=dram,
        offset=b * T * C,
        ap=[[C, L], [L * C, NB], [1, C]],
    )


def _build_nc() -> bass.Bass:
    nc = bass.Bass(trn_type="TRN2", target_bir_lowering=False, debug=False,
                   num_devices=N_CORES)
    x_d = nc.dram_tensor("x", (BS, T, C), F32, kind="ExternalInput")
    w_d = nc.dram_tensor("wts", (L + 2, _WCOLS), F32, kind="ExternalInput")
    res_d = nc.dram_tensor("res", (BS, T, C), F32, kind="ExternalOutput")
    ma_d = nc.dram_tensor("ma", (BS, T, C), F32, kind="ExternalOutput")

    with _SplitDrainTC(nc) as tc, ExitStack() as ctx:
        const = ctx.enter_context(tc.tile_pool(name="const", bufs=1))
        xbp = ctx.enter_context(tc.tile_pool(name="xb", bufs=3))
        masp = ctx.enter_context(tc.tile_pool(name="mas", bufs=3))
        resp = ctx.enter_context(tc.tile_pool(name="resb", bufs=3))
        zsp = ctx.enter_context(tc.tile_pool(name="zs", bufs=2))
        zps = ctx.enter_context(tc.tile_pool(name="zpsum", bufs=2, space="PSUM"))
        mps = ctx.enter_context(tc.tile_pool(name="mpsum", bufs=4, space="PSUM"))

        wts = const.tile([L + 2, _WCOLS], F32)
        nc.sync.dma_start(wts[:], w_d.ap())
        w0t = wts[0:L, _WC_W0T : _WC_W0T + L]
        waugt = wts[0 : L + 2, _WC_WAUGT : _WC_WAUGT + L]
        vts = [wts[0:L, _WC_VT + 14 * j : _WC_VT + 14 * (j + 1)] for j in range(NB - 1)]

        for p in range(NPAIR):
            b0 = 2 * p
            xb = xbp.tile([L + 2, NB, 2, C], F32)
            for bp in range(2):
                nc.sync.dma_start(xb[0:L, :, bp, :], _dram_row_ap(x_d, b0 + bp))

            # block-entry states: ZB = sum_j V_j @ x_blockj  -> [14, 2, C]
            zb = zps.tile([14, 2, C], F32)
            for j in range(NB - 1):
                nc.tensor.matmul(zb[:], vts[j], xb[0:L, j],
                                 start=(j == 0), stop=(j == NB - 2))
            zs = zsp.tile([14, 2, C], F32)
            nc.vector.tensor_copy(zs[:], zb[:])
            # place Z_k in partitions 96:98 under x block k (k >= 1);
            # engines need 32-aligned partition bases, DMA does not
            for k in range(1, NB):
                nc.sync.dma_start(xb[L : L + 2, k], zs[2 * (k - 1) : 2 * k])

            mas = masp.tile([L, NB, 2, C], F32)
            for k in range(NB):
                mapk = mps.tile([L, 2, C], F32, tag="mapk")
                if k == 0:
                    nc.tensor.matmul(mapk[:], w0t, xb[0:L, 0], start=True, stop=True)
                else:
                    nc.tensor.matmul(mapk[:], waugt, xb[:, k], start=True, stop=True)
                nc.scalar.copy(mas[:, k], mapk[:])

            resb = resp.tile([L, NB, 2, C], F32)
            nc.vector.tensor_sub(resb[:], xb[0:L], mas[:])

            for bp in range(2):
                nc.sync.dma_start(_dram_row_ap(ma_d, b0 + bp), mas[:, :, bp, :])
                nc.sync.dma_start(_dram_row_ap(res_d, b0 + bp), resb[:, :, bp, :])

    return nc


_NC_CACHE: bass.Bass | None = None


def _get_nc() -> bass.Bass:
    global _NC_CACHE
    if _NC_CACHE is None:
        _NC_CACHE = _build_nc()
    return _NC_CACHE


def kernel(x: np.ndarray, alpha, beta):
    x = np.ascontiguousarray(np.asarray(x, dtype=np.float32))
    assert x.shape == (B, T, C), x.shape
    wts = _pack_weights(float(alpha), float(beta))

    nc = _get_nc()
    in_maps = [
        {"x": x[i * BS : (i + 1) * BS], "wts": wts} for i in range(N_CORES)
    ]
    out = run_bass_kernel_spmd(nc, in_maps, core_ids=list(range(N_CORES)))
    res = np.concatenate([out.results[i]["res"] for i in range(N_CORES)], axis=0)
    ma = np.concatenate([out.results[i]["ma"] for i in range(N_CORES)], axis=0)
    return res, ma


# revision 13
# speedup vs baseline: 1.6428x; 1.6428x over previous
"""Trainium2 Bass kernel for DEMA (double exponential moving average) decomposition.

reference semantics (per batch row b, channel c, over time t):
    s0 = x[0], b0 = x[1] - x[0]
    for t in 1..T-1:
        s_t = alpha*x_t + (1-alpha)*(s_{t-1} + b_{t-1})
        b_t = beta*(s_t - s_{t-1}) + (1-beta)*b_{t-1}
    ma = [s0, s1, ..., s_{T-1}];  res = x - ma;  returns (res, ma)

The recurrence is linear in x, so it is restructured into dense matmuls:
time is split into NB=8 blocks of L=96.  With z_t = (s_t, b_t) and
z_t = A z_{t-1} + c x_t (A, c functions of alpha/beta only):

    ma_block0   = W0 @ x_block0
    ma_blockk   = [W | PQ] @ [x_blockk ; Z_k]       (k >= 1)
    Z_(1..7)    = sum_j V_j @ x_blockj              (block-entry states)

W0, [W|PQ], V_j are tiny constant matrices computed on the host in float64.
On-device work is fp32 TensorE matmuls + PSUM evictions + one subtract.
Z_k is copied into 2 extra SBUF partitions (96:98) below each x block so the
state contribution rides in the same matmul (K=98).

Sharding: batch dim (128) split across 8 cores (16 rows each); the
recurrence runs only over time so no cross-core communication is needed.
"""

import numpy as np
from contextlib import ExitStack

import bass_rust as _bass_rust
import concourse.bass as bass
import concourse.tile as tile
import concourse.tile_sem_assignment as _tsa
from concourse import mybir
from concourse.bass_utils import run_bass_kernel_spmd
from concourse.vector_clock import VectorClock, ScopedClock
from concourse.tile_scheduler import N_PROCS


N_CORES = 8
B, T, C = 128, 768, 256
L = 96
NB = T // L            # 8 time blocks
BS = B // N_CORES      # 16 batch rows per core
NPAIR = BS // 2        # batch rows processed in pairs
F32 = mybir.dt.float32

# packed weight tensor [98, _WCOLS]: [W0T | WAUGT | VT_0..VT_6]
_WC_W0T = 0            # rows 0:96
_WC_WAUGT = L          # rows 0:98 = [W | PQ].T
_WC_VT = 2 * L         # rows 0:96, 14 cols per j
_WCOLS = 2 * L + 14 * (NB - 1)


def _dema_matrices(alpha: float, beta: float):
    """Block-decomposition coefficient matrices, in float64."""
    a, b = float(alpha), float(beta)
    A = np.array([[1.0 - a, 1.0 - a], [-a * b, b * (1.0 - a) + (1.0 - b)]])
    c = np.array([a, a * b])
    e_s = np.array([1.0, 0.0])

    Apow = [np.eye(2)]
    for _ in range(T + 1):
        Apow.append(A @ Apow[-1])

    # generic block k>=1: z_{kL+t'} = A^{t'+1} Z_k + sum_{i'<=t'} A^{t'-i'} c x[kL+i']
    W = np.zeros((L, L))
    for tp in range(L):
        for ip in range(tp + 1):
            W[tp, ip] = (Apow[tp - ip] @ c)[0]
    PQ = np.zeros((L, 2))
    for tp in range(L):
        PQ[tp, :] = e_s @ Apow[tp + 1]
    U = np.zeros((2, L))
    for ip in range(L):
        U[:, ip] = Apow[L - 1 - ip] @ c
    T2 = Apow[L]

    # block 0: z_0 = (x_0, x_1 - x_0), ma_0 = x_0
    W0 = np.zeros((L, L))
    W0[0, 0] = 1.0
    M0 = np.array([[1.0, 0.0], [-1.0, 1.0]])
    for t in range(1, L):
        zc = Apow[t] @ M0
        W0[t, 0] += (e_s @ zc)[0]
        W0[t, 1] += (e_s @ zc)[1]
        for i in range(1, t + 1):
            W0[t, i] += (Apow[t - i] @ c)[0]
    U0 = np.zeros((2, L))
    zc = Apow[L - 1] @ M0
    U0[:, 0] += zc[:, 0]
    U0[:, 1] += zc[:, 1]
    for i in range(1, L):
        U0[:, i] += Apow[L - 1 - i] @ c

    # V_j: Z_all = sum_j V_j @ x_blockj where Z_k = sum_{j<k} T2^{k-1-j} Uj x_j
    T2pow = [np.eye(2)]
    for _ in range(NB):
        T2pow.append(T2 @ T2pow[-1])
    V = np.zeros((NB - 1, 2 * (NB - 1), L))
    for j in range(NB - 1):
        Uj = U0 if j == 0 else U
        for k in range(j + 1, NB):
            V[j, 2 * (k - 1) : 2 * k, :] = T2pow[k - 1 - j] @ Uj
    return W0, W, PQ, V


def _pack_weights(alpha: float, beta: float) -> np.ndarray:
    W0, W, PQ, V = _dema_matrices(alpha, beta)
    wts = np.zeros((L + 2, _WCOLS), dtype=np.float32)
    wts[0:L, _WC_W0T : _WC_W0T + L] = W0.T
    wts[0:L, _WC_WAUGT : _WC_WAUGT + L] = W.T
    wts[L : L + 2, _WC_WAUGT : _WC_WAUGT + L] = PQ.T
    for j in range(NB - 1):
        wts[0:L, _WC_VT + 14 * j : _WC_VT + 14 * (j + 1)] = V[j].T
    return wts


def _wait_limit(inst) -> int:
    # walrus in this container rejects >1 sync wait on several instruction
    # formats (S3_LW, DMA DIRECT2D, CTRL); keep a single wait everywhere
    return 1


class _SplitDrainTC(tile.TileContext):
    """This walrus build rejects more than a couple of sync waits per
    instruction.  After scheduling + the stock kernel-tail drain, walk every
    block and move excess waits onto injected same-engine nops placed
    immediately before the over-limit instruction (waits execute on the
    engine sequencer before dispatch, so this is semantics-preserving)."""

    def _drain_and_barrier(self, tick_clock, wait_clock):
        super()._drain_and_barrier(tick_clock, wait_clock)
        self._split_excess_waits()

    def _split_excess_waits(self):
        nc = self.nc
        cur_list = nc.cur_bb.bb.instructions if nc.cur_bb is not None else None
        for fn in nc.m.functions:
            for blk in fn.blocks:
                insts = blk.instructions
                i = 0
                while i < len(insts):
                    inst = insts[i]
                    si = getattr(inst, "sync_info", None)
                    waits = list(si.on_wait) if si is not None else []
                    limit = _wait_limit(inst)
                    if len(waits) <= limit:
                        i += 1
                        continue
                    keep = waits[:limit]
                    excess = waits[limit:]
                    nops = []
                    for j in range(0, len(excess)):
                        nop = nc.engines[inst.engine].nop(nofuse=True).ins
                        # engine.nop() appended to the current bb; relocate it
                        if cur_list is not None and cur_list and cur_list[-1] is nop:
                            cur_list.pop()
                        nop.sync_info = _bass_rust.SyncInfo(
                            on_wait=excess[j : j + 1], on_update=[]
                        )
                        nops.append(nop)
                    si.on_wait = keep
                    insts[i:i] = nops
                    i += len(nops) + 1


def _dram_row_ap(dram, b: int):
    """[t'=96 (partition), k=8, c=256] strided view of dram[b]."""
    return bass.AP(
        tensor=dram,
        offset=b * T * C,
        ap=[[C, L], [L * C, NB], [1, C]],
    )


def _build_nc() -> bass.Bass:
    nc = bass.Bass(trn_type="TRN2", target_bir_lowering=False, debug=False,
                   num_devices=N_CORES)
    x_d = nc.dram_tensor("x", (BS, T, C), F32, kind="ExternalInput")
    w_d = nc.dram_tensor("wts", (L + 2, _WCOLS), F32, kind="ExternalInput")
    res_d = nc.dram_tensor("res", (BS, T, C), F32, kind="ExternalOutput")
    ma_d = nc.dram_tensor("ma", (BS, T, C), F32, kind="ExternalOutput")

    with _SplitDrainTC(nc) as tc, ExitStack() as ctx:
        const = ctx.enter_context(tc.tile_pool(name="const", bufs=1))
        xbp = ctx.enter_context(tc.tile_pool(name="xb", bufs=3))
        masp = ctx.enter_context(tc.tile_pool(name="mas", bufs=3))
        resp = ctx.enter_context(tc.tile_pool(name="resb", bufs=3))
        zsp = ctx.enter_context(tc.tile_pool(name="zs", bufs=2))
        zps = ctx.enter_context(tc.tile_pool(name="zpsum", bufs=2, space="PSUM"))
        mps = ctx.enter_context(tc.tile_pool(name="mpsum", bufs=4, space="PSUM"))

        wts = const.tile([L + 2, _WCOLS], F32)
        nc.sync.dma_start(wts[:], w_d.ap())
        w0t = wts[0:L, _WC_W0T : _WC_W0T + L]
        waugt = wts[0 : L + 2, _WC_WAUGT : _WC_WAUGT + L]
        vts = [wts[0:L, _WC_VT + 14 * j : _WC_VT + 14 * (j + 1)] for j in range(NB - 1)]

        for p in range(NPAIR):
            b0 = 2 * p
            xb = xbp.tile([L + 2, NB, 2, C], F32)
            for bp in range(2):
                nc.sync.dma_start(xb[0:L, :, bp, :], _dram_row_ap(x_d, b0 + bp))

            # block-entry states: ZB = sum_j V_j @ x_blockj  -> [14, 2, C]
            zb = zps.tile([14, 2, C], F32)
            for j in range(NB - 1):
                nc.tensor.matmul(zb[:], vts[j], xb[0:L, j],
                                 start=(j == 0), stop=(j == NB - 2))
            zs = zsp.tile([14, 2, C], F32)
            nc.vector.tensor_copy(zs[:], zb[:])
            # place Z_k in partitions 96:98 under x block k (k >= 1);
            # engines need 32-aligned partition bases, DMA does not
            for k in range(1, NB):
                nc.sync.dma_start(xb[L : L + 2, k], zs[2 * (k - 1) : 2 * k])

            mas = masp.tile([L, NB, 2, C], F32)
            for k in range(NB):
                mapk = mps.tile([L, 2, C], F32, tag="mapk")
                if k == 0:
                    nc.tensor.matmul(mapk[:], w0t, xb[0:L, 0], start=True, stop=True)
                else:
                    nc.tensor.matmul(mapk[:], waugt, xb[:, k], start=True, stop=True)
                nc.scalar.copy(mas[:, k], mapk[:])

            resb = resp.tile([L, NB, 2, C], F32)
            nc.vector.tensor_sub(resb[:], xb[0:L], mas[:])

            # outputs go out on the ScalarE HWDGE ring so stores never queue
            # ahead of the next pair's loads on the SyncE ring
            for bp in range(2):
                nc.scalar.dma_start(_dram_row_ap(ma_d, b0 + bp), mas[:, :, bp, :])
                nc.scalar.dma_start(_dram_row_ap(res_d, b0 + bp), resb[:, :, bp, :])

    return nc


_NC_CACHE: bass.Bass | None = None


def _get_nc() -> bass.Bass:
    global _NC_CACHE
    if _NC_CACHE is None:
        _NC_CACHE = _build_nc()
    return _NC_CACHE


def kernel(x: np.ndarray, alpha, beta):
    x = np.ascontiguousarray(np.asarray(x, dtype=np.float32))
    assert x.shape == (B, T, C), x.shape
    wts = _pack_weights(float(alpha), float(beta))

    nc = _get_nc()
    in_maps = [
        {"x": x[i * BS : (i + 1) * BS], "wts": wts} for i in range(N_CORES)
    ]
    out = run_bass_kernel_spmd(nc, in_maps, core_ids=list(range(N_CORES)))
    res = np.concatenate([out.results[i]["res"] for i in range(N_CORES)], axis=0)
    ma = np.concatenate([out.results[i]["ma"] for i in range(N_CORES)], axis=0)
    return res, ma


# revision 14
# speedup vs baseline: 1.6703x; 1.0167x over previous
"""Trainium2 Bass kernel for DEMA (double exponential moving average) decomposition.

reference semantics (per batch row b, channel c, over time t):
    s0 = x[0], b0 = x[1] - x[0]
    for t in 1..T-1:
        s_t = alpha*x_t + (1-alpha)*(s_{t-1} + b_{t-1})
        b_t = beta*(s_t - s_{t-1}) + (1-beta)*b_{t-1}
    ma = [s0, s1, ..., s_{T-1}];  res = x - ma;  returns (res, ma)

The recurrence is linear in x, so it is restructured into dense matmuls:
time is split into NB=8 blocks of L=96.  With z_t = (s_t, b_t) and
z_t = A z_{t-1} + c x_t (A, c functions of alpha/beta only):

    ma_block0   = W0 @ x_block0
    ma_blockk   = [W | PQ] @ [x_blockk ; Z_k]       (k >= 1)
    Z_(1..7)    = sum_j V_j @ x_blockj              (block-entry states)

W0, [W|PQ], V_j are tiny constant matrices computed on the host in float64.
On-device work is fp32 TensorE matmuls + PSUM evictions + one subtract.
Z_k is copied into 2 extra SBUF partitions (96:98) below each x block so the
state contribution rides in the same matmul (K=98).

Sharding: batch dim (128) split across 8 cores (16 rows each); the
recurrence runs only over time so no cross-core communication is needed.
"""

import numpy as np
from contextlib import ExitStack

import bass_rust as _bass_rust
import concourse.bass as bass
import concourse.tile as tile
import concourse.tile_sem_assignment as _tsa
from concourse import mybir
from concourse.bass_utils import run_bass_kernel_spmd
from concourse.vector_clock import VectorClock, ScopedClock
from concourse.tile_scheduler import N_PROCS


N_CORES = 8
B, T, C = 128, 768, 256
L = 96
NB = T // L            # 8 time blocks
BS = B // N_CORES      # 16 batch rows per core
NPAIR = BS // 2        # batch rows processed in pairs
F32 = mybir.dt.float32

# packed weight tensor [98, _WCOLS]: [W0T | WAUGT | VT_0..VT_6]
_WC_W0T = 0            # rows 0:96
_WC_WAUGT = L          # rows 0:98 = [W | PQ].T
_WC_VT = 2 * L         # rows 0:96, 14 cols per j
_WCOLS = 2 * L + 14 * (NB - 1)


def _dema_matrices(alpha: float, beta: float):
    """Block-decomposition coefficient matrices, in float64."""
    a, b = float(alpha), float(beta)
    A = np.array([[1.0 - a, 1.0 - a], [-a * b, b * (1.0 - a) + (1.0 - b)]])
    c = np.array([a, a * b])
    e_s = np.array([1.0, 0.0])

    Apow = [np.eye(2)]
    for _ in range(T + 1):
        Apow.append(A @ Apow[-1])

    # generic block k>=1: z_{kL+t'} = A^{t'+1} Z_k + sum_{i'<=t'} A^{t'-i'} c x[kL+i']
    W = np.zeros((L, L))
    for tp in range(L):
        for ip in range(tp + 1):
            W[tp, ip] = (Apow[tp - ip] @ c)[0]
    PQ = np.zeros((L, 2))
    for tp in range(L):
        PQ[tp, :] = e_s @ Apow[tp + 1]
    U = np.zeros((2, L))
    for ip in range(L):
        U[:, ip] = Apow[L - 1 - ip] @ c
    T2 = Apow[L]

    # block 0: z_0 = (x_0, x_1 - x_0), ma_0 = x_0
    W0 = np.zeros((L, L))
    W0[0, 0] = 1.0
    M0 = np.array([[1.0, 0.0], [-1.0, 1.0]])
    for t in range(1, L):
        zc = Apow[t] @ M0
        W0[t, 0] += (e_s @ zc)[0]
        W0[t, 1] += (e_s @ zc)[1]
        for i in range(1, t + 1):
            W0[t, i] += (Apow[t - i] @ c)[0]
    U0 = np.zeros((2, L))
    zc = Apow[L - 1] @ M0
    U0[:, 0] += zc[:, 0]
    U0[:, 1] += zc[:, 1]
    for i in range(1, L):
        U0[:, i] += Apow[L - 1 - i] @ c

    # V_j: Z_all = sum_j V_j @ x_blockj where Z_k = sum_{j<k} T2^{k-1-j} Uj x_j
    T2pow = [np.eye(2)]
    for _ in range(NB):
        T2pow.append(T2 @ T2pow[-1])
    V = np.zeros((NB - 1, 2 * (NB - 1), L))
    for j in range(NB - 1):
        Uj = U0 if j == 0 else U
        for k in range(j + 1, NB):
            V[j, 2 * (k - 1) : 2 * k, :] = T2pow[k - 1 - j] @ Uj
    return W0, W, PQ, V


def _pack_weights(alpha: float, beta: float) -> np.ndarray:
    W0, W, PQ, V = _dema_matrices(alpha, beta)
    wts = np.zeros((L + 2, _WCOLS), dtype=np.float32)
    wts[0:L, _WC_W0T : _WC_W0T + L] = W0.T
    wts[0:L, _WC_WAUGT : _WC_WAUGT + L] = W.T
    wts[L : L + 2, _WC_WAUGT : _WC_WAUGT + L] = PQ.T
    for j in range(NB - 1):
        wts[0:L, _WC_VT + 14 * j : _WC_VT + 14 * (j + 1)] = V[j].T
    return wts


def _wait_limit(inst) -> int:
    # walrus in this container rejects >1 sync wait on several instruction
    # formats (S3_LW, DMA DIRECT2D, CTRL); keep a single wait everywhere
    return 1


class _SplitDrainTC(tile.TileContext):
    """This walrus build rejects more than a couple of sync waits per
    instruction.  After scheduling + the stock kernel-tail drain, walk every
    block and move excess waits onto injected same-engine nops placed
    immediately before the over-limit instruction (waits execute on the
    engine sequencer before dispatch, so this is semantics-preserving)."""

    def _drain_and_barrier(self, tick_clock, wait_clock):
        super()._drain_and_barrier(tick_clock, wait_clock)
        self._split_excess_waits()

    def _split_excess_waits(self):
        nc = self.nc
        cur_list = nc.cur_bb.bb.instructions if nc.cur_bb is not None else None
        for fn in nc.m.functions:
            for blk in fn.blocks:
                insts = blk.instructions
                i = 0
                while i < len(insts):
                    inst = insts[i]
                    si = getattr(inst, "sync_info", None)
                    waits = list(si.on_wait) if si is not None else []
                    limit = _wait_limit(inst)
                    if len(waits) <= limit:
                        i += 1
                        continue
                    keep = waits[:limit]
                    excess = waits[limit:]
                    nops = []
                    for j in range(0, len(excess)):
                        nop = nc.engines[inst.engine].nop(nofuse=True).ins
                        # engine.nop() appended to the current bb; relocate it
                        if cur_list is not None and cur_list and cur_list[-1] is nop:
                            cur_list.pop()
                        nop.sync_info = _bass_rust.SyncInfo(
                            on_wait=excess[j : j + 1], on_update=[]
                        )
                        nops.append(nop)
                    si.on_wait = keep
                    insts[i:i] = nops
                    i += len(nops) + 1


def _pair_ap(dram, p: int):
    """contiguous [96 (partition), k*b'*c = 4096] view of tiled dram[p]."""
    return bass.AP(
        tensor=dram,
        offset=p * L * NB * 2 * C,
        ap=[[NB * 2 * C, L], [1, NB * 2 * C]],
    )


def _build_nc() -> bass.Bass:
    nc = bass.Bass(trn_type="TRN2", target_bir_lowering=False, debug=False,
                   num_devices=N_CORES)
    # DRAM tensors use the SBUF tile layout [pair, t'=96, k, b', c] so every
    # DMA moves 16KB-contiguous runs per partition; the host does the
    # (cheap) permutation to/from [b, t, c] during shard/unshard.
    x_d = nc.dram_tensor("x", (NPAIR, L, NB, 2, C), F32, kind="ExternalInput")
    w_d = nc.dram_tensor("wts", (L + 2, _WCOLS), F32, kind="ExternalInput")
    res_d = nc.dram_tensor("res", (NPAIR, L, NB, 2, C), F32, kind="ExternalOutput")
    ma_d = nc.dram_tensor("ma", (NPAIR, L, NB, 2, C), F32, kind="ExternalOutput")

    with _SplitDrainTC(nc) as tc, ExitStack() as ctx:
        const = ctx.enter_context(tc.tile_pool(name="const", bufs=1))
        xbp = ctx.enter_context(tc.tile_pool(name="xb", bufs=3))
        masp = ctx.enter_context(tc.tile_pool(name="mas", bufs=3))
        resp = ctx.enter_context(tc.tile_pool(name="resb", bufs=3))
        zsp = ctx.enter_context(tc.tile_pool(name="zs", bufs=2))
        zps = ctx.enter_context(tc.tile_pool(name="zpsum", bufs=2, space="PSUM"))
        mps = ctx.enter_context(tc.tile_pool(name="mpsum", bufs=4, space="PSUM"))

        wts = const.tile([L + 2, _WCOLS], F32)
        nc.sync.dma_start(wts[:], w_d.ap())
        w0t = wts[0:L, _WC_W0T : _WC_W0T + L]
        waugt = wts[0 : L + 2, _WC_WAUGT : _WC_WAUGT + L]
        vts = [wts[0:L, _WC_VT + 14 * j : _WC_VT + 14 * (j + 1)] for j in range(NB - 1)]

        for p in range(NPAIR):
            b0 = 2 * p
            xb = xbp.tile([L + 2, NB, 2, C], F32)
            nc.sync.dma_start(
                bass.AP(tensor=xb.tensor, offset=xb.offset,
                        ap=[[xb.ap[0][0], L], [1, NB * 2 * C]]),
                _pair_ap(x_d, p),
            )

            # block-entry states: ZB = sum_j V_j @ x_blockj  -> [14, 2, C]
            zb = zps.tile([14, 2, C], F32)
            for j in range(NB - 1):
                nc.tensor.matmul(zb[:], vts[j], xb[0:L, j],
                                 start=(j == 0), stop=(j == NB - 2))
            zs = zsp.tile([14, 2, C], F32)
            nc.vector.tensor_copy(zs[:], zb[:])
            # place Z_k in partitions 96:98 under x block k (k >= 1);
            # engines need 32-aligned partition bases, DMA does not
            for k in range(1, NB):
                nc.sync.dma_start(xb[L : L + 2, k], zs[2 * (k - 1) : 2 * k])

            mas = masp.tile([L, NB, 2, C], F32)
            for k in range(NB):
                mapk = mps.tile([L, 2, C], F32, tag="mapk")
                if k == 0:
                    nc.tensor.matmul(mapk[:], w0t, xb[0:L, 0], start=True, stop=True)
                else:
                    nc.tensor.matmul(mapk[:], waugt, xb[:, k], start=True, stop=True)
                nc.scalar.copy(mas[:, k], mapk[:])

            resb = resp.tile([L, NB, 2, C], F32)
            nc.vector.tensor_sub(resb[:], xb[0:L], mas[:])

            # outputs go out on the ScalarE HWDGE ring so stores never queue
            # ahead of the next pair's loads on the SyncE ring
            nc.scalar.dma_start(_pair_ap(ma_d, p), mas[:])
            nc.scalar.dma_start(_pair_ap(res_d, p), resb[:])

    return nc


_NC_CACHE: bass.Bass | None = None


def _get_nc() -> bass.Bass:
    global _NC_CACHE
    if _NC_CACHE is None:
        _NC_CACHE = _build_nc()
    return _NC_CACHE


def _tile_layout(x_shard: np.ndarray) -> np.ndarray:
    """[BS, T, C] -> [NPAIR, L, NB, 2, C] tile-contiguous layout."""
    v = x_shard.reshape(NPAIR, 2, NB, L, C)
    return np.ascontiguousarray(v.transpose(0, 3, 2, 1, 4))


def _untile_layout(t: np.ndarray) -> np.ndarray:
    """[NPAIR, L, NB, 2, C] -> [BS, T, C]."""
    return t.transpose(0, 3, 2, 1, 4).reshape(BS, T, C)


def kernel(x: np.ndarray, alpha, beta):
    x = np.asarray(x, dtype=np.float32)
    assert x.shape == (B, T, C), x.shape
    wts = _pack_weights(float(alpha), float(beta))

    nc = _get_nc()
    in_maps = [
        {"x": _tile_layout(x[i * BS : (i + 1) * BS]), "wts": wts}
        for i in range(N_CORES)
    ]
    out = run_bass_kernel_spmd(nc, in_maps, core_ids=list(range(N_CORES)))
    res = np.concatenate(
        [_untile_layout(out.results[i]["res"]) for i in range(N_CORES)], axis=0
    )
    ma = np.concatenate(
        [_untile_layout(out.results[i]["ma"]) for i in range(N_CORES)], axis=0
    )
    return res, ma


# revision 19
# speedup vs baseline: 1.9059x; 1.1411x over previous
"""Trainium2 Bass kernel for DEMA (double exponential moving average) decomposition.

reference semantics (per batch row b, channel c, over time t):
    s0 = x[0], b0 = x[1] - x[0]
    for t in 1..T-1:
        s_t = alpha*x_t + (1-alpha)*(s_{t-1} + b_{t-1})
        b_t = beta*(s_t - s_{t-1}) + (1-beta)*b_{t-1}
    ma = [s0, s1, ..., s_{T-1}];  res = x - ma;  returns (res, ma)

The recurrence is linear in x, so it is restructured into dense matmuls:
time is split into NB=8 blocks of L=96.  With z_t = (s_t, b_t) and
z_t = A z_{t-1} + c x_t (A, c functions of alpha/beta only), one fused
constant stationary per block computes outputs AND the carry state:

    [ma_block0 ; Z_1]     = [W0 ; U0]        @ x_block0          (98 x 96)
    [ma_blockk ; Z_{k+1}] = [[W, PQ],[U, T2]] @ [x_blockk ; Z_k]  (98 x 98)

All matrices are computed on the host in float64 from alpha/beta.  The
carry Z_k is copied into 2 extra SBUF partitions (96:98) under x block k,
so each block is ONE float32r TensorE matmul; partition bases 96 are
32-aligned so engine copies of the carry rows are legal.

Sharding: batch dim (128) split across 8 cores (16 rows each); the
recurrence runs only over time so no cross-core communication is needed.
"""

import numpy as np
from contextlib import ExitStack

import bass_rust as _bass_rust
import concourse.bass as bass
import concourse.tile as tile
import concourse.tile_sem_assignment as _tsa
from concourse import mybir
from concourse.bass_utils import run_bass_kernel_spmd
from concourse.vector_clock import VectorClock, ScopedClock
from concourse.tile_scheduler import N_PROCS


N_CORES = 8
B, T, C = 128, 768, 256
L = 96
NB = T // L            # 8 time blocks
BS = B // N_CORES      # 16 batch rows per core
NPAIR = BS // 2        # batch rows processed in pairs
F32 = mybir.dt.float32
F32R = mybir.dt.float32r

# packed weight tensor [98, _WCOLS]: [W0AUG.T | WAUG2.T]
_WC_W0AUGT = 0         # [96, 98] in rows 0:96
_WC_WAUG2T = L + 2     # [98, 98]
_WCOLS = 2 * (L + 2)


def _dema_matrices(alpha: float, beta: float):
    """Block-decomposition coefficient matrices, in float64."""
    a, b = float(alpha), float(beta)
    A = np.array([[1.0 - a, 1.0 - a], [-a * b, b * (1.0 - a) + (1.0 - b)]])
    c = np.array([a, a * b])
    e_s = np.array([1.0, 0.0])

    Apow = [np.eye(2)]
    for _ in range(T + 1):
        Apow.append(A @ Apow[-1])

    # generic block k>=1: z_{kL+t'} = A^{t'+1} Z_k + sum_{i'<=t'} A^{t'-i'} c x[kL+i']
    W = np.zeros((L, L))
    for tp in range(L):
        for ip in range(tp + 1):
            W[tp, ip] = (Apow[tp - ip] @ c)[0]
    PQ = np.zeros((L, 2))
    for tp in range(L):
        PQ[tp, :] = e_s @ Apow[tp + 1]
    U = np.zeros((2, L))
    for ip in range(L):
        U[:, ip] = Apow[L - 1 - ip] @ c
    T2 = Apow[L]

    # block 0: z_0 = (x_0, x_1 - x_0), ma_0 = x_0
    W0 = np.zeros((L, L))
    W0[0, 0] = 1.0
    M0 = np.array([[1.0, 0.0], [-1.0, 1.0]])
    for t in range(1, L):
        zc = Apow[t] @ M0
        W0[t, 0] += (e_s @ zc)[0]
        W0[t, 1] += (e_s @ zc)[1]
        for i in range(1, t + 1):
            W0[t, i] += (Apow[t - i] @ c)[0]
    U0 = np.zeros((2, L))
    zc = Apow[L - 1] @ M0
    U0[:, 0] += zc[:, 0]
    U0[:, 1] += zc[:, 1]
    for i in range(1, L):
        U0[:, i] += Apow[L - 1 - i] @ c

    W0AUG = np.vstack([W0, U0])                    # [98, 96]
    WAUG2 = np.block([[W, PQ], [U, T2]])           # [98, 98]
    return W0AUG, WAUG2


def _pack_weights(alpha: float, beta: float) -> np.ndarray:
    W0AUG, WAUG2 = _dema_matrices(alpha, beta)
    wts = np.zeros((L + 2, _WCOLS), dtype=np.float32)
    wts[0:L, _WC_W0AUGT : _WC_W0AUGT + L + 2] = W0AUG.T
    wts[:, _WC_WAUG2T : _WC_WAUG2T + L + 2] = WAUG2.T
    return wts


def _wait_limit(inst) -> int:
    # walrus in this container rejects >1 sync wait on several instruction
    # formats (S3_LW, DMA DIRECT2D, CTRL); keep a single wait everywhere
    return 1


class _SplitDrainTC(tile.TileContext):
    """This walrus build rejects more than a couple of sync waits per
    instruction.  After scheduling + the stock kernel-tail drain, walk every
    block and move excess waits onto injected same-engine nops placed
    immediately before the over-limit instruction (waits execute on the
    engine sequencer before dispatch, so this is semantics-preserving)."""

    def _drain_and_barrier(self, tick_clock, wait_clock):
        super()._drain_and_barrier(tick_clock, wait_clock)
        self._split_excess_waits()

    def _split_excess_waits(self):
        nc = self.nc
        cur_list = nc.cur_bb.bb.instructions if nc.cur_bb is not None else None
        for fn in nc.m.functions:
            for blk in fn.blocks:
                insts = blk.instructions
                i = 0
                while i < len(insts):
                    inst = insts[i]
                    si = getattr(inst, "sync_info", None)
                    waits = list(si.on_wait) if si is not None else []
                    limit = _wait_limit(inst)
                    if len(waits) <= limit:
                        i += 1
                        continue
                    keep = waits[:limit]
                    excess = waits[limit:]
                    nops = []
                    for j in range(0, len(excess)):
                        nop = nc.engines[inst.engine].nop(nofuse=True).ins
                        # engine.nop() appended to the current bb; relocate it
                        if cur_list is not None and cur_list and cur_list[-1] is nop:
                            cur_list.pop()
                        nop.sync_info = _bass_rust.SyncInfo(
                            on_wait=excess[j : j + 1], on_update=[]
                        )
                        nops.append(nop)
                    si.on_wait = keep
                    insts[i:i] = nops
                    i += len(nops) + 1


def _pair_ap(dram, p: int):
    """contiguous [96 (partition), k*b'*c = 4096] view of tiled dram[p]."""
    return bass.AP(
        tensor=dram,
        offset=p * L * NB * 2 * C,
        ap=[[NB * 2 * C, L], [1, NB * 2 * C]],
    )


def _build_nc() -> bass.Bass:
    nc = bass.Bass(trn_type="TRN2", target_bir_lowering=False, debug=False,
                   num_devices=N_CORES)
    # DRAM tensors use the SBUF tile layout [pair, t'=96, k, b', c] so every
    # DMA moves 16KB-contiguous runs per partition; the host does the
    # (cheap) permutation to/from [b, t, c] during shard/unshard.
    x_d = nc.dram_tensor("x", (NPAIR, L, NB, 2, C), F32R, kind="ExternalInput")
    w_d = nc.dram_tensor("wts", (L + 2, _WCOLS), F32R, kind="ExternalInput")
    res_d = nc.dram_tensor("res", (NPAIR, L, NB, 2, C), F32, kind="ExternalOutput")
    ma_d = nc.dram_tensor("ma", (NPAIR, L, NB, 2, C), F32, kind="ExternalOutput")

    with _SplitDrainTC(nc) as tc, ExitStack() as ctx:
        const = ctx.enter_context(tc.tile_pool(name="const", bufs=1))
        xbp = ctx.enter_context(tc.tile_pool(name="xb", bufs=3))
        masp = ctx.enter_context(tc.tile_pool(name="mas", bufs=3))
        resp = ctx.enter_context(tc.tile_pool(name="resb", bufs=3))
        mps = ctx.enter_context(tc.tile_pool(name="mpsum", bufs=4, space="PSUM"))

        wts = const.tile([L + 2, _WCOLS], F32R)
        nc.sync.dma_start(wts[:], w_d.ap())
        w0augt = wts[0:L, _WC_W0AUGT : _WC_W0AUGT + L + 2]
        waug2t = wts[:, _WC_WAUG2T : _WC_WAUG2T + L + 2]

        for p in range(NPAIR):
            xb = xbp.tile([L + 2, NB, 2, C], F32R)
            nc.sync.dma_start(
                bass.AP(tensor=xb.tensor, offset=xb.offset,
                        ap=[[xb.ap[0][0], L], [1, NB * 2 * C]]),
                _pair_ap(x_d, p),
            )

            # per-block fused matmul: rows 0:96 = ma, rows 96:98 = carry Z
            mas = masp.tile([L + 2, NB, 2, C], F32)
            for k in range(NB):
                mapk = mps.tile([L + 2, 2, C], F32, tag="mapk")
                if k == 0:
                    nc.tensor.matmul(mapk[:], w0augt, xb[0:L, 0],
                                     start=True, stop=True)
                else:
                    nc.tensor.matmul(mapk[:], waug2t, xb[:, k],
                                     start=True, stop=True)
                nc.scalar.copy(mas[:, k], mapk[:])
                if k + 1 < NB:
                    # carry rows ride at partition base 96 (32-aligned)
                    nc.vector.tensor_copy(xb[L : L + 2, k + 1], mas[L : L + 2, k])

            resb = resp.tile([L, NB, 2, C], F32)
            if p % 2 == 0:
                nc.vector.tensor_sub(resb[:], xb[0:L].bitcast(F32), mas[0:L])
            else:
                nc.gpsimd.tensor_sub(resb[:], xb[0:L].bitcast(F32), mas[0:L])

            # outputs go out on the ScalarE HWDGE ring so stores never queue
            # ahead of the next pair's loads on the SyncE ring
            nc.scalar.dma_start(_pair_ap(ma_d, p), mas[0:L])
            nc.scalar.dma_start(_pair_ap(res_d, p), resb[:])

    return nc


_NC_CACHE: bass.Bass | None = None


def _get_nc() -> bass.Bass:
    global _NC_CACHE
    if _NC_CACHE is None:
        _NC_CACHE = _build_nc()
    return _NC_CACHE


def _tile_layout(x_shard: np.ndarray) -> np.ndarray:
    """[BS, T, C] -> [NPAIR, L, NB, 2, C] tile-contiguous layout."""
    v = x_shard.reshape(NPAIR, 2, NB, L, C)
    return np.ascontiguousarray(v.transpose(0, 3, 2, 1, 4))


def _untile_layout(t: np.ndarray) -> np.ndarray:
    """[NPAIR, L, NB, 2, C] -> [BS, T, C]."""
    return t.transpose(0, 3, 2, 1, 4).reshape(BS, T, C)


def kernel(x: np.ndarray, alpha, beta):
    x = np.asarray(x, dtype=np.float32)
    assert x.shape == (B, T, C), x.shape
    wts = _pack_weights(float(alpha), float(beta))

    nc = _get_nc()
    in_maps = [
        {"x": _tile_layout(x[i * BS : (i + 1) * BS]), "wts": wts}
        for i in range(N_CORES)
    ]
    out = run_bass_kernel_spmd(nc, in_maps, core_ids=list(range(N_CORES)))
    res = np.concatenate(
        [_untile_layout(out.results[i]["res"]) for i in range(N_CORES)], axis=0
    )
    ma = np.concatenate(
        [_untile_layout(out.results[i]["ma"]) for i in range(N_CORES)], axis=0
    )
    return res, ma


# revision 21
# speedup vs baseline: 2.2652x; 1.1885x over previous
"""Trainium2 Bass kernel for DEMA (double exponential moving average) decomposition.

reference semantics (per batch row b, channel c, over time t):
    s0 = x[0], b0 = x[1] - x[0]
    for t in 1..T-1:
        s_t = alpha*x_t + (1-alpha)*(s_{t-1} + b_{t-1})
        b_t = beta*(s_t - s_{t-1}) + (1-beta)*b_{t-1}
    ma = [s0, s1, ..., s_{T-1}];  res = x - ma;  returns (res, ma)

The recurrence is linear in x, so it is restructured into dense matmuls:
time is split into NB=8 blocks of L=96.  With z_t = (s_t, b_t) and
z_t = A z_{t-1} + c x_t (A, c functions of alpha/beta only), one fused
constant stationary per block computes outputs AND the carry state:

    [ma_block0 ; Z_1]     = [W0 ; U0]        @ x_block0          (98 x 96)
    [ma_blockk ; Z_{k+1}] = [[W, PQ],[U, T2]] @ [x_blockk ; Z_k]  (98 x 98)

All matrices are computed on the host in float64 from alpha/beta.  The
carry Z_k is copied into 2 extra SBUF partitions (96:98) under x block k,
so each block is ONE float32r TensorE matmul; partition bases 96 are
32-aligned so engine copies of the carry rows are legal.

Sharding: batch dim (128) split across 8 cores (16 rows each); the
recurrence runs only over time so no cross-core communication is needed.
"""

import numpy as np
from contextlib import ExitStack

import bass_rust as _bass_rust
import concourse.bass as bass
import concourse.tile as tile
import concourse.tile_sem_assignment as _tsa
from concourse import mybir
from concourse.bass_utils import run_bass_kernel_spmd
from concourse.vector_clock import VectorClock, ScopedClock
from concourse.tile_scheduler import N_PROCS


N_CORES = 8
B, T, C = 128, 768, 256
L = 96
NB = T // L            # 8 time blocks
BS = B // N_CORES      # 16 batch rows per core
NPAIR = BS // 2        # batch rows processed in pairs
F32 = mybir.dt.float32
F32R = mybir.dt.float32r

# packed weight tensor [98, _WCOLS]: [W0AUG.T | WAUG2.T]
_WC_W0AUGT = 0         # [96, 98] in rows 0:96
_WC_WAUG2T = L + 2     # [98, 98]
_WCOLS = 2 * (L + 2)


def _dema_matrices(alpha: float, beta: float):
    """Block-decomposition coefficient matrices, in float64."""
    a, b = float(alpha), float(beta)
    A = np.array([[1.0 - a, 1.0 - a], [-a * b, b * (1.0 - a) + (1.0 - b)]])
    c = np.array([a, a * b])
    e_s = np.array([1.0, 0.0])

    Apow = [np.eye(2)]
    for _ in range(T + 1):
        Apow.append(A @ Apow[-1])

    # generic block k>=1: z_{kL+t'} = A^{t'+1} Z_k + sum_{i'<=t'} A^{t'-i'} c x[kL+i']
    W = np.zeros((L, L))
    for tp in range(L):
        for ip in range(tp + 1):
            W[tp, ip] = (Apow[tp - ip] @ c)[0]
    PQ = np.zeros((L, 2))
    for tp in range(L):
        PQ[tp, :] = e_s @ Apow[tp + 1]
    U = np.zeros((2, L))
    for ip in range(L):
        U[:, ip] = Apow[L - 1 - ip] @ c
    T2 = Apow[L]

    # block 0: z_0 = (x_0, x_1 - x_0), ma_0 = x_0
    W0 = np.zeros((L, L))
    W0[0, 0] = 1.0
    M0 = np.array([[1.0, 0.0], [-1.0, 1.0]])
    for t in range(1, L):
        zc = Apow[t] @ M0
        W0[t, 0] += (e_s @ zc)[0]
        W0[t, 1] += (e_s @ zc)[1]
        for i in range(1, t + 1):
            W0[t, i] += (Apow[t - i] @ c)[0]
    U0 = np.zeros((2, L))
    zc = Apow[L - 1] @ M0
    U0[:, 0] += zc[:, 0]
    U0[:, 1] += zc[:, 1]
    for i in range(1, L):
        U0[:, i] += Apow[L - 1 - i] @ c

    W0AUG = np.vstack([W0, U0])                    # [98, 96]
    WAUG2 = np.block([[W, PQ], [U, T2]])           # [98, 98]
    return W0AUG, WAUG2


def _pack_weights(alpha: float, beta: float) -> np.ndarray:
    W0AUG, WAUG2 = _dema_matrices(alpha, beta)
    wts = np.zeros((L + 2, _WCOLS), dtype=np.float32)
    wts[0:L, _WC_W0AUGT : _WC_W0AUGT + L + 2] = W0AUG.T
    wts[:, _WC_WAUG2T : _WC_WAUG2T + L + 2] = WAUG2.T
    return wts


def _wait_limit(inst) -> int:
    # walrus in this container rejects >1 sync wait on several instruction
    # formats (S3_LW, DMA DIRECT2D, CTRL); keep a single wait everywhere
    return 1


class _SplitDrainTC(tile.TileContext):
    """This walrus build rejects more than a couple of sync waits per
    instruction.  After scheduling + the stock kernel-tail drain, walk every
    block and move excess waits onto injected same-engine nops placed
    immediately before the over-limit instruction (waits execute on the
    engine sequencer before dispatch, so this is semantics-preserving)."""

    def _drain_and_barrier(self, tick_clock, wait_clock):
        super()._drain_and_barrier(tick_clock, wait_clock)
        self._split_excess_waits()

    def _split_excess_waits(self):
        nc = self.nc
        cur_list = nc.cur_bb.bb.instructions if nc.cur_bb is not None else None
        for fn in nc.m.functions:
            for blk in fn.blocks:
                insts = blk.instructions
                i = 0
                while i < len(insts):
                    inst = insts[i]
                    si = getattr(inst, "sync_info", None)
                    waits = list(si.on_wait) if si is not None else []
                    limit = _wait_limit(inst)
                    if len(waits) <= limit:
                        i += 1
                        continue
                    keep = waits[:limit]
                    excess = waits[limit:]
                    nops = []
                    for j in range(0, len(excess)):
                        nop = nc.engines[inst.engine].nop(nofuse=True).ins
                        # engine.nop() appended to the current bb; relocate it
                        if cur_list is not None and cur_list and cur_list[-1] is nop:
                            cur_list.pop()
                        nop.sync_info = _bass_rust.SyncInfo(
                            on_wait=excess[j : j + 1], on_update=[]
                        )
                        nops.append(nop)
                    si.on_wait = keep
                    insts[i:i] = nops
                    i += len(nops) + 1


def _pair_ap(dram, p: int):
    """contiguous [96 (partition), k*b'*c = 4096] view of tiled dram[p]."""
    return bass.AP(
        tensor=dram,
        offset=p * L * NB * 2 * C,
        ap=[[NB * 2 * C, L], [1, NB * 2 * C]],
    )


def _build_nc() -> bass.Bass:
    nc = bass.Bass(trn_type="TRN2", target_bir_lowering=False, debug=False,
                   num_devices=N_CORES)
    # DRAM tensors use the SBUF tile layout [pair, t'=96, k, b', c] so every
    # DMA moves 16KB-contiguous runs per partition; the host does the
    # (cheap) permutation to/from [b, t, c] during shard/unshard.
    x_d = nc.dram_tensor("x", (NPAIR, L, NB, 2, C), F32R, kind="ExternalInput")
    w_d = nc.dram_tensor("wts", (L + 2, _WCOLS), F32R, kind="ExternalInput")
    res_d = nc.dram_tensor("res", (NPAIR, L, NB, 2, C), F32, kind="ExternalOutput")
    ma_d = nc.dram_tensor("ma", (NPAIR, L, NB, 2, C), F32, kind="ExternalOutput")

    with _SplitDrainTC(nc) as tc, ExitStack() as ctx:
        const = ctx.enter_context(tc.tile_pool(name="const", bufs=1))
        xbp = ctx.enter_context(tc.tile_pool(name="xb", bufs=5))
        masp = ctx.enter_context(tc.tile_pool(name="mas", bufs=5))
        resp = ctx.enter_context(tc.tile_pool(name="resb", bufs=2))
        mps = ctx.enter_context(tc.tile_pool(name="mpsum", bufs=4, space="PSUM"))

        wts = const.tile([L + 2, _WCOLS], F32R)
        nc.sync.dma_start(wts[:], w_d.ap())
        w0augt = wts[0:L, _WC_W0AUGT : _WC_W0AUGT + L + 2]
        waug2t = wts[:, _WC_WAUG2T : _WC_WAUG2T + L + 2]

        # Two waves of 4 interleaved pair-chains: PE executes in emission
        # order, so blocks are emitted k-outer / pair-inner — each chain's
        # evict+carry-copy latency is hidden behind the other pairs' matmuls.
        WAVE = 4
        for w in range(NPAIR // WAVE):
            pairs = range(WAVE * w, WAVE * (w + 1))
            xbs, mass = {}, {}
            for p in pairs:
                xb = xbp.tile([L + 2, NB, 2, C], F32R, tag="xb", name=f"xb{p}")
                nc.sync.dma_start(
                    bass.AP(tensor=xb.tensor, offset=xb.offset,
                            ap=[[xb.ap[0][0], L], [1, NB * 2 * C]]),
                    _pair_ap(x_d, p),
                )
                xbs[p] = xb
                mass[p] = masp.tile([L + 2, NB, 2, C], F32, tag="mas", name=f"mas{p}")

            for k in range(NB):
                for p in pairs:
                    xb, mas = xbs[p], mass[p]
                    mapk = mps.tile([L + 2, 2, C], F32, tag="mapk")
                    if k == 0:
                        nc.tensor.matmul(mapk[:], w0augt, xb[0:L, 0],
                                         start=True, stop=True)
                    else:
                        nc.tensor.matmul(mapk[:], waug2t, xb[:, k],
                                         start=True, stop=True)
                    nc.scalar.copy(mas[:, k], mapk[:])
                    if k + 1 < NB:
                        # carry rows ride at partition base 96 (32-aligned)
                        nc.vector.tensor_copy(xb[L : L + 2, k + 1],
                                              mas[L : L + 2, k])

            for p in pairs:
                xb, mas = xbs[p], mass[p]
                resb = resp.tile([L, NB, 2, C], F32)
                if p % 2 == 0:
                    nc.vector.tensor_sub(resb[:], xb[0:L].bitcast(F32), mas[0:L])
                else:
                    nc.gpsimd.tensor_sub(resb[:], xb[0:L].bitcast(F32), mas[0:L])
                # outputs ride the ScalarE HWDGE ring so stores never queue
                # ahead of the next wave's loads on the SyncE ring
                nc.scalar.dma_start(_pair_ap(ma_d, p), mas[0:L])
                nc.scalar.dma_start(_pair_ap(res_d, p), resb[:])

    return nc


_NC_CACHE: bass.Bass | None = None


def _get_nc() -> bass.Bass:
    global _NC_CACHE
    if _NC_CACHE is None:
        _NC_CACHE = _build_nc()
    return _NC_CACHE


def _tile_layout(x_shard: np.ndarray) -> np.ndarray:
    """[BS, T, C] -> [NPAIR, L, NB, 2, C] tile-contiguous layout."""
    v = x_shard.reshape(NPAIR, 2, NB, L, C)
    return np.ascontiguousarray(v.transpose(0, 3, 2, 1, 4))


def _untile_layout(t: np.ndarray) -> np.ndarray:
    """[NPAIR, L, NB, 2, C] -> [BS, T, C]."""
    return t.transpose(0, 3, 2, 1, 4).reshape(BS, T, C)


def kernel(x: np.ndarray, alpha, beta):
    x = np.asarray(x, dtype=np.float32)
    assert x.shape == (B, T, C), x.shape
    wts = _pack_weights(float(alpha), float(beta))

    nc = _get_nc()
    in_maps = [
        {"x": _tile_layout(x[i * BS : (i + 1) * BS]), "wts": wts}
        for i in range(N_CORES)
    ]
    out = run_bass_kernel_spmd(nc, in_maps, core_ids=list(range(N_CORES)))
    res = np.concatenate(
        [_untile_layout(out.results[i]["res"]) for i in range(N_CORES)], axis=0
    )
    ma = np.concatenate(
        [_untile_layout(out.results[i]["ma"]) for i in range(N_CORES)], axis=0
    )
    return res, ma
